# revision 1
# baseline (speedup 1.0000x reference)
"""AdaFace loss on 8 TRN2 NeuronCores.

Math: for non-label columns, cos(arccos(clip(x))) == clip(x), so the
scaled logit matrix is just 64*x except at the single label column per
row.  Since |64*x| <= 64 and e^64 < f32 max, the row logsumexp can be
computed without max-subtraction: the device does the memory-bound pass
S[b] = sum_j exp(64*x[b,j]) and the tiny per-row label correction +
cross-entropy runs on the host in float64.

Sharding: 512 rows x 100000 cols -> 4 row-groups (128 rows, fills all
SBUF partitions) x 2 column-halves (50000 cols) = 8 cores, 25.6MB/core.
"""

import contextlib
import math

import numpy as np

import concourse.bass as bass
import concourse.mybir as mybir
from concourse.bass_utils import run_bass_kernel_spmd

B, C = 512, 100000
N_CORES = 8
P = 128                      # rows per core (partition dim)
COL_HALVES = 2
COLS = C // COL_HALVES       # 50000 columns per core
NT = 12                      # tiles per core
# Tile widths: big uniform mains + a geometric tail.
# Measured on silicon (K-fold NEFF slope timing): each DMA instruction
# costs ~300-460ns of real stream time that the cost model does not
# charge, so few big DMAs beat many small ones (NT=8-12 ran 52-67us/pass
# vs 78-90us for NT=50).  The tail tapers (ratio <= ~1.6, the silicon
# hide-constraint ACT(w_prev) <= DMA(w_next)) so the exposure after the
# DMA stream ends is ~2.2us instead of 5.6us for uniform 6250s.
WIDTHS = [6250] * 6 + [3500, 2750, 2150, 1700, 1350, 1050]
assert sum(WIDTHS) == COLS and len(WIDTHS) == NT
WMAX = max(WIDTHS)           # 6250 (slot stride)
OFFS = [sum(WIDTHS[:i]) for i in range(NT)]

H_PARAM = 0.333
S_PARAM = 64.0
M_PARAM = 0.4
EPS = 1e-06

_nc_cache = None


def _build():
    global _nc_cache
    if _nc_cache is not None:
        return _nc_cache
    nc = bass.Bass()
    f32 = mybir.dt.float32
    x = nc.declare_dram_parameter("x", [P, COLS], f32, isOutput=False)
    out = nc.declare_dram_parameter("out", [P, NT], f32, isOutput=True)
    NBUF = 6                 # 6 x 6250 f32 = 150KB/partition of SBUF
    with (
        nc.sbuf_tensor([P, NBUF * WMAX], f32) as tbuf,
        nc.sbuf_tensor([P, WMAX], f32) as scratch,
        nc.sbuf_tensor([P, NT], f32) as acc,
        nc.semaphore("act_sem") as act_sem,
        nc.semaphore("out_sem") as out_sem,
    ):
        # One DMA-completion semaphore per buffer slot: a DMA's 16
        # per-engine increments are unordered across engines, so a
        # cumulative threshold on one shared semaphore can be satisfied
        # by a mixture of increments from different DMAs (observed as
        # sub-8-row stale reads).  Per-slot semaphores make each wait
        # count only its own tile's DMA — exact, since a slot's next DMA
        # cannot issue until the ACT consuming the current one completes.
        with contextlib.ExitStack() as stack:
            dsem = [
                stack.enter_context(nc.semaphore(f"dsem{s}")) for s in range(NBUF)
            ]
            with nc.Block() as block:

                @block.sync
                def _(sync):
                    for i, w in enumerate(WIDTHS):
                        if i >= NBUF:
                            # the ACT that freed this slot implies its DMA done
                            sync.wait_ge(act_sem, i - NBUF + 1)
                        s0 = (i % NBUF) * WMAX
                        sync.dma_start(
                            out=tbuf[:, s0 : s0 + w],
                            in_=x[:, OFFS[i] : OFFS[i] + w],
                        ).then_inc(dsem[i % NBUF], 16)
                    sync.wait_ge(act_sem, NT)
                    # walrus requires sync info on every DGE DMA, so the
                    # final DMA increments out_sem even though nothing waits
                    sync.dma_start(out=out[:], in_=acc[:]).then_inc(out_sem, 16)

                @block.scalar
                def _(scalar):
                    for i, w in enumerate(WIDTHS):
                        scalar.wait_ge(dsem[i % NBUF], 16 * (i // NBUF + 1))
                        s0 = (i % NBUF) * WMAX
                        scalar.activation(
                            scratch[:, :w],
                            tbuf[:, s0 : s0 + w],
                            mybir.ActivationFunctionType.Exp,
                            bias=0.0,
                            scale=S_PARAM,
                            accum_out=acc[:, i : i + 1],
                        ).then_inc(act_sem, 1)

    _nc_cache = nc
    return nc


def kernel(logits, norms, labels):
    logits = np.asarray(logits, dtype=np.float32)
    norms = np.asarray(norms, dtype=np.float32)
    labels_i = np.asarray(labels).astype(np.int64)

    nc = _build()
    in_maps = []
    for c in range(N_CORES):
        g, h = divmod(c, COL_HALVES)
        shard = np.ascontiguousarray(
            logits[g * P : (g + 1) * P, h * COLS : (h + 1) * COLS]
        )
        in_maps.append({"x": shard})
    res = run_bass_kernel_spmd(nc, in_maps, core_ids=list(range(N_CORES)))

    # S[b] = sum_j exp(64 * logits[b, j]) summed across the two column halves
    S = np.zeros(B, dtype=np.float64)
    for c in range(N_CORES):
        g, h = divmod(c, COL_HALVES)
        S[g * P : (g + 1) * P] += res.results[c]["out"].astype(np.float64).sum(axis=1)

    # Host epilogue (all [512]-sized, float64)
    safe_norms = np.clip(norms.astype(np.float64), 0.001, 100.0).reshape(-1)
    mean = safe_norms.mean()
    std = safe_norms.std(ddof=1)
    margin_scaler = np.clip((safe_norms - mean) / (std + EPS) * H_PARAM, -1.0, 1.0)
    g_angular = -M_PARAM * margin_scaler
    g_add = M_PARAM + M_PARAM * margin_scaler

    x_lab = logits[np.arange(B), labels_i].astype(np.float64)
    cosc = np.clip(x_lab, -1.0 + EPS, 1.0 - EPS)
    theta = np.arccos(cosc)
    theta_m = np.clip(theta + g_angular, EPS, math.pi - EPS)
    q = S_PARAM * (np.cos(theta_m) - g_add)

    # swap the label column's plain term for the margin-adjusted one
    S_corr = S - np.exp(S_PARAM * x_lab) + np.exp(q)
    S_corr = np.maximum(S_corr, np.finfo(np.float64).tiny)
    nll = np.log(S_corr) - q
    return np.array(nll.mean(), dtype=np.float32)

